# revision 7
# baseline (speedup 1.0000x reference)
"""GAT-style message passing (edge softmax + weighted aggregation) on 8 NeuronCores.

Algorithm (mathematically identical to the reference, up to fp reassociation):
  hs = feat_src @ W_src.T + b_src            (per node, 8 heads x 16 dims)
  el = <hs, a_l>, er = <hd, a_r>             (per node, per head)
  w_e = exp(leakyrelu(el[src]+er[dst]))      (per edge; softmax max-subtraction
                                              dropped -- logits are O(1), exp is
                                              safe, and the max cancels exactly
                                              in the normalization)
  out[v] = (sum_e w_e * hs_nb[src]) / (sum_e w_e) + b_src
           (b_src factors out because softmax weights sum to 1 per (v, head))

Device mapping:
  - "Z table" in HBM per core: row n = [hs_nb (128 cols, head-minor) | el (8) | er (8)]
    built by a replicated bf16 GEMM over all nodes (TensorE), bias-free.
  - Edges sorted by dst; dst nodes split into 8 contiguous per-core ranges with
    ~equal edge counts.  Per core, consecutive dsts are greedily packed into
    blocks of <= SW dsts and <= TPB*128 edge slots.
  - Per 128-edge tile: rows of the Z table are fetched with indirect DMA
    (src-indexed 136-col reads + dst-indexed 8-col reads), attention weights
    computed on VectorE/ScalarE, and a host-built one-hot matrix S maps the
    weighted messages onto the block's dst slots via a TensorE matmul that
    accumulates in PSUM (segment-sum as matmul).  The un-normalized denominator
    rides along as 8 extra columns.
  - Per block: normalize (reciprocal + multiply, converting the head-minor
    layout back to head-major), add b_src, and scatter rows to the core's
    output range with indirect DMA.
"""

import sys

for _p in ("/opt/trn_rl_repo",):
    if _p not in sys.path:
        sys.path.insert(0, _p)

import numpy as np
import ml_dtypes

import concourse.bass as bass
import concourse.bacc as bacc
import concourse.mybir as mybir
import concourse.tile as tile
from concourse.bass import IndirectOffsetOnAxis
from concourse.bass_utils import run_bass_kernel_spmd

BF16 = ml_dtypes.bfloat16
P = 128  # SBUF partitions / edge slots per tile


class Cfg:
    def __init__(self, n_nodes, d_in, kh, dh, n_cores, sw, tpb, bg, neg_slope=0.2):
        assert d_in % P == 0
        self.n_nodes = n_nodes
        self.d_in = d_in
        self.kh = kh  # heads
        self.dh = dh  # dims per head
        self.c = kh * dh  # feature cols (128)
        self.n_cores = n_cores
        self.sw = sw  # max dsts per block (S width)
        self.tpb = tpb  # tiles (of 128 edge slots) per block
        self.slots = tpb * P
        self.bg = bg  # blocks per processing batch
        self.neg_slope = neg_slope
        self.kc = d_in // P  # K chunks for GEMM
        self.zgc = self.c + kh  # gathered cols per src row (hs + el)
        self.zcols = self.zgc + kh  # stored row cols (hs + el + er), no pad
        assert self.c == P, "flush/permute APs assume C == 128"


FULL_CFG = Cfg(n_nodes=50000, d_in=256, kh=8, dh=16, n_cores=8, sw=64, tpb=8, bg=4)


# ----------------------------------------------------------------------------
# Host-side preprocessing
# ----------------------------------------------------------------------------

def _head_minor_perm(cfg):
    # z col c (c < C) holds original channel (c % kh) * dh + (c // kh)
    c = np.arange(cfg.c)
    return (c % cfg.kh) * cfg.dh + (c // cfg.kh)


def build_weights(cfg, W_src, b_src, W_dst, b_dst, attn):
    kh, dh, din = cfg.kh, cfg.dh, cfg.d_in
    a_l = attn[:, :dh]  # (kh, dh)
    a_r = attn[:, dh:]
    perm = _head_minor_perm(cfg)

    W_el = np.einsum("kd,kdi->ki", a_l, W_src.reshape(kh, dh, din))  # (kh, din)
    W_er = np.einsum("kd,kdi->ki", a_r, W_dst.reshape(kh, dh, din))
    c_el = np.einsum("kd,kd->k", a_l, b_src.reshape(kh, dh))
    c_er = np.einsum("kd,kd->k", a_r, b_dst.reshape(kh, dh))

    # src-side rhs: cols [hs head-minor (C) | el (kh)]
    Wz = np.concatenate([W_src[perm].T, W_el.T], axis=1)  # (din, C+kh)
    Wz = Wz.reshape(cfg.kc, P, cfg.zgc).astype(BF16)
    Wer = W_er.T.reshape(cfg.kc, P, kh).astype(BF16)  # (kc, P, kh)

    c_vec = (c_el + c_er).astype(np.float32)
    c_rep = np.tile(c_vec[None, :], (P, 1)).astype(BF16)  # (P, kh)
    b_rep = np.tile(b_src[None, :], (P, 1)).astype(np.float32)  # (P, C) head-major
    return Wz, Wer, c_rep, b_rep


def build_schedule(cfg, src_idx, dst_idx):
    """Sort edges by dst, split dsts across cores, pack blocks, build per-core
    index/one-hot arrays (uniform shapes across cores)."""
    E = src_idx.shape[0]
    n = cfg.n_nodes
    order = np.argsort(dst_idx, kind="stable")
    ssrc = src_idx[order].astype(np.int64)
    counts = np.bincount(dst_idx, minlength=n).astype(np.int64)
    starts = np.zeros(n + 1, dtype=np.int64)
    np.cumsum(counts, out=starts[1:])

    # split dst range into n_cores contiguous chunks with ~equal edges
    bounds = [0]
    for ci in range(1, cfg.n_cores):
        target = E * ci // cfg.n_cores
        d = int(np.searchsorted(starts, target))
        d = max(bounds[-1], min(d, n))
        bounds.append(d)
    bounds.append(n)

    per_core_blocks = []  # list of list of (d0, ndst, edge_start, nedges)
    for ci in range(cfg.n_cores):
        d0, d1 = bounds[ci], bounds[ci + 1]
        blocks = []
        d = d0
        while d < d1:
            bstart = d
            used = 0
            while d < d1 and (d - bstart) < cfg.sw and used + counts[d] <= cfg.slots:
                used += int(counts[d])
                d += 1
            assert d > bstart, f"dst {d} degree {counts[d]} exceeds {cfg.slots}"
            blocks.append((bstart, d - bstart, int(starts[bstart]), used))
        per_core_blocks.append(blocks)

    nb = max(len(b) for b in per_core_blocks)
    nbg = -(-nb // cfg.bg)
    nb = nbg * cfg.bg
    nt = nb * cfg.tpb  # tiles per core

    rng = max(bounds[ci + 1] - bounds[ci] for ci in range(cfg.n_cores))
    trash = rng  # local trash row index in the output table

    idx_z = np.zeros((cfg.n_cores, P, nt), dtype=np.int32)
    idx_er = np.zeros((cfg.n_cores, P, nt), dtype=np.int32)
    s_arr = np.zeros((cfg.n_cores, nb, cfg.tpb, P, cfg.sw), dtype=BF16)
    out_idx = np.full((cfg.n_cores, P, nb), trash, dtype=np.int32)

    for ci in range(cfg.n_cores):
        d0c = bounds[ci]
        for bi, (bstart, ndst, estart, nedges) in enumerate(per_core_blocks[ci]):
            if nedges > 0:
                sl = np.arange(nedges)
                t = sl // P
                p = sl % P
                gcol = bi * cfg.tpb + t
                esrc = ssrc[estart : estart + nedges]
                # dst-local slot for each edge (edges sorted by dst)
                dloc = np.repeat(
                    np.arange(ndst), counts[bstart : bstart + ndst]
                ).astype(np.int64)
                idx_z[ci, p, gcol] = esrc
                # er gather reads the same Z-table rows, dst-indexed
                idx_er[ci, p, gcol] = np.repeat(
                    np.arange(bstart, bstart + ndst),
                    counts[bstart : bstart + ndst],
                )
                s_arr[ci, bi, t, p, dloc] = 1.0
            out_idx[ci, :ndst, bi] = np.arange(bstart - d0c, bstart - d0c + ndst)

    # rearrange S to SBUF-friendly layout: (n_cores, nbg, P, bg*tpb*sw)
    s_arr = (
        s_arr.reshape(cfg.n_cores, nbg, cfg.bg, cfg.tpb, P, cfg.sw)
        .transpose(0, 1, 4, 2, 3, 5)
        .reshape(cfg.n_cores, nbg, P, cfg.bg * cfg.tpb * cfg.sw)
    )
    s_arr = np.ascontiguousarray(s_arr)
    meta = dict(bounds=bounds, nb=nb, nbg=nbg, nt=nt, rng=rng)
    return idx_z, idx_er, s_arr, out_idx, meta


# ----------------------------------------------------------------------------
# Device program
# ----------------------------------------------------------------------------

def build_program(cfg, nb, nbg, rng_sz):
    n, kh, c, kc = cfg.n_nodes, cfg.kh, cfg.c, cfg.kc
    zgc, zcols, sw, tpb, bg = cfg.zgc, cfg.zcols, cfg.sw, cfg.tpb, cfg.bg
    nt = nb * tpb
    gpt = bg * tpb  # gather tiles per batch
    bf = mybir.dt.bfloat16
    f32 = mybir.dt.float32
    i32 = mybir.dt.int32

    nc = bacc.Bacc(
        "TRN2", target_bir_lowering=False, debug=False, num_devices=cfg.n_cores
    )

    fsrcT = nc.dram_tensor("fsrcT", [kc, P, n], bf, kind="ExternalInput")
    fdstT = nc.dram_tensor("fdstT", [kc, P, n], bf, kind="ExternalInput")
    wz_d = nc.dram_tensor("wz", [kc, P, zgc], bf, kind="ExternalInput")
    wer_d = nc.dram_tensor("wer", [kc, P, kh], bf, kind="ExternalInput")
    crep_d = nc.dram_tensor("crep", [P, kh], bf, kind="ExternalInput")
    brep_d = nc.dram_tensor("brep", [P, c], f32, kind="ExternalInput")
    idxz_d = nc.dram_tensor("idxz", [P, nt], i32, kind="ExternalInput")
    idxer_d = nc.dram_tensor("idxer", [P, nt], i32, kind="ExternalInput")
    s_d = nc.dram_tensor("s", [nbg, P, bg * tpb * sw], bf, kind="ExternalInput")
    oidx_d = nc.dram_tensor("oidx", [P, nb], i32, kind="ExternalInput")

    z_d = nc.dram_tensor("ztab", [n, zcols], bf, kind="Internal")
    out_d = nc.dram_tensor("out", [rng_sz + 1, c], f32, kind="ExternalOutput")

    ntile_n = -(-n // P)

    with tile.TileContext(nc) as tc:
        with (
            tc.tile_pool(name="consts", bufs=1) as cpool,
            tc.tile_pool(name="psum", bufs=4, space="PSUM") as ppool,
        ):
            # ---------------- Phase A: node GEMMs -> Z table ----------------
            wz_sb = [
                cpool.tile([P, zgc], bf, name=f"wz_sb{k}", tag=f"wz_sb{k}")
                for k in range(kc)
            ]
            wer_sb = [
                cpool.tile([P, kh], bf, name=f"wer_sb{k}", tag=f"wer_sb{k}")
                for k in range(kc)
            ]
            crep_sb = cpool.tile([P, kh], bf, name="crep_sb")
            brep_sb = cpool.tile([P, c], f32, name="brep_sb")
            for k in range(kc):
                nc.sync.dma_start(out=wz_sb[k][:], in_=wz_d[k])
                nc.sync.dma_start(out=wer_sb[k][:], in_=wer_d[k])
            nc.sync.dma_start(out=crep_sb[:], in_=crep_d[:])
            nc.sync.dma_start(out=brep_sb[:], in_=brep_d[:])

            with tc.tile_pool(name="gemm", bufs=3) as gpool:
                for it in range(ntile_n):
                    n0 = it * P
                    nn = min(P, n - n0)
                    fs = gpool.tile([P, kc, P], bf, name="fs", tag="fs")
                    fd = gpool.tile([P, kc, P], bf, name="fd", tag="fd")
                    # (P, kc, nn): partition = feat-dim chunk rows
                    nc.sync.dma_start(
                        out=fs[:, :, :nn],
                        in_=bass.AP(fsrcT.ap().tensor, n0, [[n, P], [P * n, kc], [1, nn]]),
                    )
                    nc.sync.dma_start(
                        out=fd[:, :, :nn],
                        in_=bass.AP(fdstT.ap().tensor, n0, [[n, P], [P * n, kc], [1, nn]]),
                    )
                    ps = ppool.tile([P, zcols], f32, name="ps", tag="ps")
                    for k in range(kc):
                        nc.tensor.matmul(
                            ps[:nn, 0:zgc],
                            lhsT=fs[:, k, :nn],
                            rhs=wz_sb[k][:],
                            start=(k == 0),
                            stop=(k == kc - 1),
                        )
                    for k in range(kc):
                        nc.tensor.matmul(
                            ps[:nn, zgc:zcols],
                            lhsT=fd[:, k, :nn],
                            rhs=wer_sb[k][:],
                            start=(k == 0),
                            stop=(k == kc - 1),
                        )
                    zsb = gpool.tile([P, zcols], bf, name="zsb", tag="zsb")
                    nc.scalar.copy(zsb[:nn], ps[:nn])
                    nc.sync.dma_start(out=z_d[n0 : n0 + nn, :], in_=zsb[:nn])

            tc.strict_bb_all_engine_barrier()

            # ---------------- Phase B: edge processing ----------------------
            with tc.tile_pool(name="edge", bufs=2) as epool:
                for g in range(nbg):
                    g0 = g * gpt  # first gather tile of batch
                    izt = epool.tile([P, gpt], i32, name="izt", tag="izt")
                    iet = epool.tile([P, gpt], i32, name="iet", tag="iet")
                    nc.sync.dma_start(out=izt[:], in_=idxz_d[:, g0 : g0 + gpt])
                    nc.sync.dma_start(out=iet[:], in_=idxer_d[:, g0 : g0 + gpt])
                    ssb = epool.tile([P, bg * tpb * sw], bf, name="ssb", tag="ssb")
                    nc.sync.dma_start(out=ssb[:], in_=s_d[g])
                    oit = epool.tile([P, bg], i32, name="oit", tag="oit")
                    nc.sync.dma_start(out=oit[:], in_=oidx_d[:, g * bg : (g + 1) * bg])

                    # HW indirect DMA supports one index per partition per
                    # call -> loop over gather tiles (G=1 each)
                    zg = epool.tile([P, gpt, zgc], bf, name="zg", tag="zg")
                    for gt in range(gpt):
                        nc.gpsimd.indirect_dma_start(
                            out=zg[:, gt, :],
                            out_offset=None,
                            in_=z_d[:, :],
                            in_offset=IndirectOffsetOnAxis(ap=izt[:, gt : gt + 1], axis=0),
                        )
                    erg = epool.tile([P, gpt, kh], bf, name="erg", tag="erg")
                    for gt in range(gpt):
                        nc.gpsimd.indirect_dma_start(
                            out=erg[:, gt, :],
                            out_offset=None,
                            in_=z_d[:, :],
                            in_offset=IndirectOffsetOnAxis(ap=iet[:, gt : gt + 1], axis=0),
                            element_offset=zgc,
                        )

                    zg_t = zg.tensor
                    zg_off = zg.offset

                    # logits x = el + er + c
                    x = epool.tile([P, gpt * kh], bf, name="x", tag="x")
                    el_ap = bass.AP(zg_t, zg_off + c, [[gpt * zgc, P], [zgc, gpt], [1, kh]])
                    nc.vector.tensor_tensor(
                        out=x[:], in0=el_ap, in1=erg[:], op=mybir.AluOpType.add
                    )
                    c_ap = bass.AP(
                        crep_sb.tensor, crep_sb.offset, [[kh, P], [0, gpt], [1, kh]]
                    )
                    nc.vector.tensor_tensor(
                        out=x[:], in0=x[:], in1=c_ap, op=mybir.AluOpType.add
                    )
                    # w = exp(leakyrelu(x)) = max(exp(x), exp(neg*x))
                    e1 = epool.tile([P, gpt * kh], bf, name="e1", tag="e1")
                    e2 = epool.tile([P, gpt * kh], bf, name="e2", tag="e2")
                    nc.scalar.activation(e1[:], x[:], mybir.ActivationFunctionType.Exp)
                    nc.scalar.activation(
                        e2[:], x[:], mybir.ActivationFunctionType.Exp,
                        scale=float(cfg.neg_slope),
                    )
                    msb = epool.tile([P, gpt, zgc], bf, name="msb", tag="msb")
                    m_t = msb.tensor
                    m_off = msb.offset
                    w_ap = bass.AP(m_t, m_off + c, [[gpt * zgc, P], [zgc, gpt], [1, kh]])
                    nc.vector.tensor_tensor(
                        out=w_ap, in0=e1[:], in1=e2[:], op=mybir.AluOpType.max
                    )
                    # weighted messages M = hs * w (head-minor broadcast)
                    m_out = bass.AP(m_t, m_off, [[gpt * zgc, P], [zgc, gpt], [1, c]])
                    hs_ap = bass.AP(zg_t, zg_off, [[gpt * zgc, P], [zgc, gpt], [1, c]])
                    wb_ap = bass.AP(
                        m_t, m_off + c, [[gpt * zgc, P], [zgc, gpt], [0, cfg.dh], [1, kh]]
                    )
                    nc.vector.tensor_tensor(
                        out=m_out, in0=hs_ap, in1=wb_ap, op=mybir.AluOpType.mult
                    )

                    # segment-sum matmuls + per-block flush
                    stg = epool.tile([sw, bg, zgc], f32, name="stg", tag="stg")
                    for b in range(bg):
                        pb = ppool.tile([sw, zgc], f32, name="pb", tag="pb")
                        for t in range(tpb):
                            nc.tensor.matmul(
                                pb[:, :],
                                lhsT=ssb[:, (b * tpb + t) * sw : (b * tpb + t + 1) * sw],
                                rhs=msb[:, b * tpb + t, :],
                                start=(t == 0),
                                stop=(t == tpb - 1),
                            )
                        nc.scalar.copy(stg[:, b, :], pb[:, :])

                    st_t = stg.tensor
                    st_off = stg.offset
                    den = bass.AP(st_t, st_off + c, [[bg * zgc, sw], [zgc, bg], [1, kh]])
                    nc.vector.tensor_scalar_max(den, den, 1e-30)
                    rcp = epool.tile([sw, bg * kh], f32, name="rcp", tag="rcp")
                    nc.vector.reciprocal(rcp[:], den)

                    outp = epool.tile([sw, bg, c], f32, name="outp", tag="outp")
                    o_t = outp.tensor
                    o_off = outp.offset
                    # numerator (head-minor d*kh+h) -> out col h*dh+d (head-major)
                    num_ap = bass.AP(
                        st_t, st_off, [[bg * zgc, sw], [zgc, bg], [kh, cfg.dh], [1, kh]]
                    )
                    out_ap = bass.AP(
                        o_t, o_off, [[bg * c, sw], [c, bg], [1, cfg.dh], [cfg.dh, kh]]
                    )
                    rcp_ap = bass.AP(
                        rcp.tensor, rcp.offset, [[bg * kh, sw], [kh, bg], [0, cfg.dh], [1, kh]]
                    )
                    nc.vector.tensor_tensor(
                        out=out_ap, in0=num_ap, in1=rcp_ap, op=mybir.AluOpType.mult
                    )
                    b_ap = bass.AP(
                        brep_sb.tensor, brep_sb.offset, [[c, sw], [0, bg], [1, c]]
                    )
                    ofl = bass.AP(o_t, o_off, [[bg * c, sw], [1, bg * c]])
                    nc.vector.tensor_tensor(
                        out=ofl, in0=ofl, in1=b_ap, op=mybir.AluOpType.add
                    )
                    for b in range(bg):
                        nc.gpsimd.indirect_dma_start(
                            out=out_d[:, :],
                            out_offset=IndirectOffsetOnAxis(
                                ap=oit[:sw, b : b + 1], axis=0
                            ),
                            in_=outp[:, b, :],
                            in_offset=None,
                        )

    nc.compile()
    return nc


# ----------------------------------------------------------------------------
# Entry point
# ----------------------------------------------------------------------------

def _run(cfg, inputs, trace=False):
    feat_src = np.asarray(inputs["feat_src"], dtype=np.float32)
    feat_dst = np.asarray(inputs["feat_dst"], dtype=np.float32)
    W_src = np.asarray(inputs["W_src"], dtype=np.float32)
    b_src = np.asarray(inputs["b_src"], dtype=np.float32)
    W_dst = np.asarray(inputs["W_dst"], dtype=np.float32)
    b_dst = np.asarray(inputs["b_dst"], dtype=np.float32)
    attn = np.asarray(inputs["attn"], dtype=np.float32)
    src_idx = np.asarray(inputs["src_idx"]).astype(np.int64)
    dst_idx = np.asarray(inputs["dst_idx"]).astype(np.int64)

    Wz, Wer, c_rep, b_rep = build_weights(cfg, W_src, b_src, W_dst, b_dst, attn)
    idx_z, idx_er, s_arr, out_idx, meta = build_schedule(cfg, src_idx, dst_idx)
    nb, nbg, rng = meta["nb"], meta["nbg"], meta["rng"]

    fsrcT = np.ascontiguousarray(
        feat_src.T.reshape(cfg.kc, P, cfg.n_nodes)
    ).astype(BF16)
    fdstT = np.ascontiguousarray(
        feat_dst.T.reshape(cfg.kc, P, cfg.n_nodes)
    ).astype(BF16)

    nc = build_program(cfg, nb, nbg, rng)

    in_maps = []
    for ci in range(cfg.n_cores):
        in_maps.append(
            {
                "fsrcT": fsrcT,
                "fdstT": fdstT,
                "wz": Wz,
                "wer": Wer,
                "crep": c_rep,
                "brep": b_rep,
                "idxz": idx_z[ci],
                "idxer": idx_er[ci],
                "s": s_arr[ci],
                "oidx": out_idx[ci],
            }
        )

    res = run_bass_kernel_spmd(
        nc, in_maps, core_ids=list(range(cfg.n_cores)), trace=trace
    )

    bounds = meta["bounds"]
    out = np.empty((cfg.n_nodes, cfg.c), dtype=np.float32)
    for ci in range(cfg.n_cores):
        d0, d1 = bounds[ci], bounds[ci + 1]
        out[d0:d1] = res.results[ci]["out"][: d1 - d0]
    # zero-in-degree nodes aggregate nothing (the softmax-weights-sum-to-1
    # bias fold only holds when there is at least one incoming edge)
    deg = np.bincount(dst_idx, minlength=cfg.n_nodes)
    out[deg == 0] = 0.0
    return out, res


def kernel(**inputs) -> np.ndarray:
    out, _ = _run(FULL_CFG, inputs, trace=False)
    return out


# revision 14
# speedup vs baseline: 1.2719x; 1.2719x over previous
"""GAT-style message passing (edge softmax + weighted aggregation) on 8 NeuronCores.

Algorithm (mathematically identical to the reference, up to fp reassociation):
  hs = feat_src @ W_src.T + b_src            (per node, 8 heads x 16 dims)
  el = <hs, a_l>, er = <hd, a_r>             (per node, per head)
  w_e = exp(leakyrelu(el[src]+er[dst]))      (per edge; softmax max-subtraction
                                              dropped -- logits are O(1), exp is
                                              safe, and the max cancels exactly
                                              in the normalization)
  out[v] = (sum_e w_e * hs_nb[src]) / (sum_e w_e) + b_src
           (b_src factors out because softmax weights sum to 1 per (v, head))

Device mapping:
  - "Z table" in HBM per core: row n = [hs_nb (128 cols, head-minor) | el (8) | er (8)]
    built by a replicated bf16 GEMM over all nodes (TensorE), bias-free.
  - Edges sorted by dst; dst nodes split into 8 contiguous per-core ranges with
    ~equal edge counts.  Per core, consecutive dsts are greedily packed into
    blocks of <= SW dsts and <= TPB*128 edge slots.
  - Per 128-edge tile: rows of the Z table are fetched with indirect DMA
    (src-indexed 136-col reads, one row per partition per call -- the HW DGE
    contract); per-block er values are fetched once per block (<=64 rows) and
    expanded to edge slots via a transposed one-hot matmul on TensorE.
    Attention weights are computed on VectorE/ScalarE, and a host-built
    one-hot matrix S maps the weighted messages onto the block's dst slots
    via a TensorE matmul that accumulates in PSUM (segment-sum as matmul).
    The un-normalized softmax denominator rides along as 8 extra columns.
  - Per block: normalize (reciprocal + multiply, converting the head-minor
    layout back to head-major), add b_src, and scatter rows to the core's
    output range with indirect DMA.
"""

import sys

for _p in ("/opt/trn_rl_repo",):
    if _p not in sys.path:
        sys.path.insert(0, _p)

import numpy as np
import ml_dtypes

import concourse.bass as bass
import concourse.bacc as bacc
import concourse.mybir as mybir
import concourse.tile as tile
from concourse.bass import IndirectOffsetOnAxis
from concourse.bass_utils import run_bass_kernel_spmd

BF16 = ml_dtypes.bfloat16
P = 128  # SBUF partitions / edge slots per tile


class Cfg:
    def __init__(self, n_nodes, d_in, kh, dh, n_cores, sw, tpb, bg, neg_slope=0.2):
        assert d_in % P == 0
        self.n_nodes = n_nodes
        self.d_in = d_in
        self.kh = kh  # heads
        self.dh = dh  # dims per head
        self.c = kh * dh  # feature cols (128)
        self.n_cores = n_cores
        self.sw = sw  # max dsts per block (S width)
        self.tpb = tpb  # tiles (of 128 edge slots) per block
        self.slots = tpb * P
        self.bg = bg  # blocks per processing batch
        self.neg_slope = neg_slope
        self.kc = d_in // P  # K chunks for GEMM
        self.zgc = self.c + kh  # gathered cols per src row (hs + el)
        self.zcols = self.zgc + kh  # stored row cols (hs + el + er), no pad
        assert self.c == P, "flush/permute APs assume C == 128"


FULL_CFG = Cfg(n_nodes=50000, d_in=256, kh=8, dh=16, n_cores=8, sw=64, tpb=8, bg=4)


# ----------------------------------------------------------------------------
# Host-side preprocessing
# ----------------------------------------------------------------------------

def _head_minor_perm(cfg):
    # z col c (c < C) holds original channel (c % kh) * dh + (c // kh)
    c = np.arange(cfg.c)
    return (c % cfg.kh) * cfg.dh + (c // cfg.kh)


def build_weights(cfg, W_src, b_src, W_dst, b_dst, attn):
    kh, dh, din = cfg.kh, cfg.dh, cfg.d_in
    a_l = attn[:, :dh]  # (kh, dh)
    a_r = attn[:, dh:]
    perm = _head_minor_perm(cfg)

    W_el = np.einsum("kd,kdi->ki", a_l, W_src.reshape(kh, dh, din))  # (kh, din)
    W_er = np.einsum("kd,kdi->ki", a_r, W_dst.reshape(kh, dh, din))
    c_el = np.einsum("kd,kd->k", a_l, b_src.reshape(kh, dh))
    c_er = np.einsum("kd,kd->k", a_r, b_dst.reshape(kh, dh))

    # src-side rhs: cols [hs head-minor (C) | el (kh)]
    Wz = np.concatenate([W_src[perm].T, W_el.T], axis=1)  # (din, C+kh)
    Wz = Wz.reshape(cfg.kc, P, cfg.zgc).astype(BF16)
    Wer = W_er.T.reshape(cfg.kc, P, kh).astype(BF16)  # (kc, P, kh)

    c_vec = (c_el + c_er).astype(np.float32)
    c_rep = np.tile(c_vec[None, :], (P, 1)).astype(BF16)  # (P, kh)
    b_rep = np.tile(b_src[None, :], (P, 1)).astype(np.float32)  # (P, C) head-major
    return Wz, Wer, c_rep, b_rep


def build_schedule(cfg, src_idx, dst_idx):
    """Sort edges by dst, split dsts across cores, pack blocks, build per-core
    index/one-hot arrays (uniform shapes across cores)."""
    E = src_idx.shape[0]
    n = cfg.n_nodes
    order = np.argsort(dst_idx, kind="stable")
    ssrc = src_idx[order].astype(np.int64)
    counts = np.bincount(dst_idx, minlength=n).astype(np.int64)
    starts = np.zeros(n + 1, dtype=np.int64)
    np.cumsum(counts, out=starts[1:])

    # split dst range into n_cores contiguous chunks with ~equal edges
    bounds = [0]
    for ci in range(1, cfg.n_cores):
        target = E * ci // cfg.n_cores
        d = int(np.searchsorted(starts, target))
        d = max(bounds[-1], min(d, n))
        bounds.append(d)
    bounds.append(n)

    per_core_blocks = []  # list of list of (d0, ndst, edge_start, nedges)
    for ci in range(cfg.n_cores):
        d0, d1 = bounds[ci], bounds[ci + 1]
        blocks = []
        d = d0
        while d < d1:
            bstart = d
            used = 0
            while d < d1 and (d - bstart) < cfg.sw and used + counts[d] <= cfg.slots:
                used += int(counts[d])
                d += 1
            assert d > bstart, f"dst {d} degree {counts[d]} exceeds {cfg.slots}"
            blocks.append((bstart, d - bstart, int(starts[bstart]), used))
        per_core_blocks.append(blocks)

    nb = max(len(b) for b in per_core_blocks)
    nbg = -(-nb // cfg.bg)
    nb = nbg * cfg.bg
    nt = nb * cfg.tpb  # tiles per core

    rng = max(bounds[ci + 1] - bounds[ci] for ci in range(cfg.n_cores))
    trash = rng  # local trash row index in the output table

    idx_z = np.zeros((cfg.n_cores, P, nt), dtype=np.int32)
    idx_er = np.zeros((cfg.n_cores, P, nt), dtype=np.int32)
    s_arr = np.zeros((cfg.n_cores, nb, cfg.tpb, P, cfg.sw), dtype=BF16)
    out_idx = np.full((cfg.n_cores, P, nb), trash, dtype=np.int32)
    gdst = np.zeros((cfg.n_cores, P, nb), dtype=np.int32)

    for ci in range(cfg.n_cores):
        d0c = bounds[ci]
        for bi, (bstart, ndst, estart, nedges) in enumerate(per_core_blocks[ci]):
            if nedges > 0:
                sl = np.arange(nedges)
                t = sl // P
                p = sl % P
                gcol = bi * cfg.tpb + t
                esrc = ssrc[estart : estart + nedges]
                # dst-local slot for each edge (edges sorted by dst)
                dloc = np.repeat(
                    np.arange(ndst), counts[bstart : bstart + ndst]
                ).astype(np.int64)
                idx_z[ci, p, gcol] = esrc
                # er gather reads the same Z-table rows, dst-indexed
                idx_er[ci, p, gcol] = np.repeat(
                    np.arange(bstart, bstart + ndst),
                    counts[bstart : bstart + ndst],
                )
                s_arr[ci, bi, t, p, dloc] = 1.0
            out_idx[ci, :ndst, bi] = np.arange(bstart - d0c, bstart - d0c + ndst)
            gdst[ci, :ndst, bi] = np.arange(bstart, bstart + ndst)

    # transposed one-hot for the er-expansion matmul: (n_cores, nbg, sw, bg*tpb*P)
    st_arr = np.ascontiguousarray(
        s_arr.reshape(cfg.n_cores, nbg, cfg.bg, cfg.tpb, P, cfg.sw)
        .transpose(0, 1, 5, 2, 3, 4)
        .reshape(cfg.n_cores, nbg, cfg.sw, cfg.bg * cfg.tpb * P)
    )
    # rearrange S to SBUF-friendly layout: (n_cores, nbg, P, bg*tpb*sw)
    s_arr = (
        s_arr.reshape(cfg.n_cores, nbg, cfg.bg, cfg.tpb, P, cfg.sw)
        .transpose(0, 1, 4, 2, 3, 5)
        .reshape(cfg.n_cores, nbg, P, cfg.bg * cfg.tpb * cfg.sw)
    )
    s_arr = np.ascontiguousarray(s_arr)
    meta = dict(bounds=bounds, nb=nb, nbg=nbg, nt=nt, rng=rng)
    return idx_z, idx_er, s_arr, st_arr, gdst, out_idx, meta


# ----------------------------------------------------------------------------
# Device program
# ----------------------------------------------------------------------------

def build_program(cfg, nb, nbg, rng_sz):
    n, kh, c, kc = cfg.n_nodes, cfg.kh, cfg.c, cfg.kc
    zgc, zcols, sw, tpb, bg = cfg.zgc, cfg.zcols, cfg.sw, cfg.tpb, cfg.bg
    nt = nb * tpb
    gpt = bg * tpb  # gather tiles per batch
    bf = mybir.dt.bfloat16
    f32 = mybir.dt.float32
    i32 = mybir.dt.int32

    nc = bacc.Bacc(
        "TRN2", target_bir_lowering=False, debug=False, num_devices=cfg.n_cores
    )

    fsrcT = nc.dram_tensor("fsrcT", [kc, P, n], bf, kind="ExternalInput")
    fdstT = nc.dram_tensor("fdstT", [kc, P, n], bf, kind="ExternalInput")
    wz_d = nc.dram_tensor("wz", [kc, P, zgc], bf, kind="ExternalInput")
    wer_d = nc.dram_tensor("wer", [kc, P, kh], bf, kind="ExternalInput")
    crep_d = nc.dram_tensor("crep", [P, kh], bf, kind="ExternalInput")
    brep_d = nc.dram_tensor("brep", [P, c], f32, kind="ExternalInput")
    idxz_d = nc.dram_tensor("idxz", [P, nt], i32, kind="ExternalInput")
    gdst_d = nc.dram_tensor("gdst", [P, nb], i32, kind="ExternalInput")
    s_d = nc.dram_tensor("s", [nbg, P, bg * tpb * sw], bf, kind="ExternalInput")
    st_d = nc.dram_tensor("st", [nbg, sw, bg * tpb * P], bf, kind="ExternalInput")
    oidx_d = nc.dram_tensor("oidx", [P, nb], i32, kind="ExternalInput")

    z_d = nc.dram_tensor("ztab", [n, zcols], bf, kind="Internal")
    out_d = nc.dram_tensor("out", [rng_sz + 1, c], f32, kind="ExternalOutput")

    ntile_n = -(-n // P)

    with tile.TileContext(nc) as tc:
        with (
            tc.tile_pool(name="consts", bufs=1) as cpool,
            tc.tile_pool(name="psum", bufs=4, space="PSUM") as ppool,
        ):
            # ---------------- Phase A: node GEMMs -> Z table ----------------
            wz_sb = [
                cpool.tile([P, zgc], bf, name=f"wz_sb{k}", tag=f"wz_sb{k}")
                for k in range(kc)
            ]
            wer_sb = [
                cpool.tile([P, kh], bf, name=f"wer_sb{k}", tag=f"wer_sb{k}")
                for k in range(kc)
            ]
            crep_sb = cpool.tile([P, kh], bf, name="crep_sb")
            brep_sb = cpool.tile([P, c], f32, name="brep_sb")
            for k in range(kc):
                nc.sync.dma_start(out=wz_sb[k][:], in_=wz_d[k])
                nc.sync.dma_start(out=wer_sb[k][:], in_=wer_d[k])
            nc.sync.dma_start(out=crep_sb[:], in_=crep_d[:])
            nc.sync.dma_start(out=brep_sb[:], in_=brep_d[:])

            with tc.tile_pool(name="gemm", bufs=3) as gpool:
                for it in range(ntile_n):
                    n0 = it * P
                    nn = min(P, n - n0)
                    fs = gpool.tile([P, kc, P], bf, name="fs", tag="fs")
                    fd = gpool.tile([P, kc, P], bf, name="fd", tag="fd")
                    # (P, kc, nn): partition = feat-dim chunk rows
                    nc.sync.dma_start(
                        out=fs[:, :, :nn],
                        in_=bass.AP(fsrcT.ap().tensor, n0, [[n, P], [P * n, kc], [1, nn]]),
                    )
                    nc.sync.dma_start(
                        out=fd[:, :, :nn],
                        in_=bass.AP(fdstT.ap().tensor, n0, [[n, P], [P * n, kc], [1, nn]]),
                    )
                    ps = ppool.tile([P, zcols], f32, name="ps", tag="ps", bufs=2)
                    for k in range(kc):
                        nc.tensor.matmul(
                            ps[:nn, 0:zgc],
                            lhsT=fs[:, k, :nn],
                            rhs=wz_sb[k][:],
                            start=(k == 0),
                            stop=(k == kc - 1),
                        )
                    for k in range(kc):
                        nc.tensor.matmul(
                            ps[:nn, zgc:zcols],
                            lhsT=fd[:, k, :nn],
                            rhs=wer_sb[k][:],
                            start=(k == 0),
                            stop=(k == kc - 1),
                        )
                    zsb = gpool.tile([P, zcols], bf, name="zsb", tag="zsb")
                    nc.scalar.copy(zsb[:nn], ps[:nn])
                    nc.sync.dma_start(out=z_d[n0 : n0 + nn, :], in_=zsb[:nn])

            tc.strict_bb_all_engine_barrier()

            # ---------------- Phase B: edge processing ----------------------
            with tc.tile_pool(name="edge", bufs=2) as epool:
                for g in range(nbg):
                    g0 = g * gpt  # first gather tile of batch
                    izt = epool.tile([P, gpt], i32, name="izt", tag="izt")
                    nc.sync.dma_start(out=izt[:], in_=idxz_d[:, g0 : g0 + gpt])
                    ssb = epool.tile([P, bg * tpb * sw], bf, name="ssb", tag="ssb")
                    nc.sync.dma_start(out=ssb[:], in_=s_d[g])
                    stsb = epool.tile([sw, bg * tpb * P], bf, name="stsb", tag="stsb")
                    nc.sync.dma_start(out=stsb[:], in_=st_d[g])
                    oit = epool.tile([P, bg], i32, name="oit", tag="oit")
                    nc.sync.dma_start(out=oit[:], in_=oidx_d[:, g * bg : (g + 1) * bg])
                    gdt = epool.tile([P, bg], i32, name="gdt", tag="gdt")
                    nc.sync.dma_start(out=gdt[:], in_=gdst_d[:, g * bg : (g + 1) * bg])

                    # HW indirect DMA supports one index per partition per
                    # call -> loop over gather tiles (G=1 each)
                    zg = epool.tile([P, gpt, zgc], bf, name="zg", tag="zg")
                    for gt in range(gpt):
                        nc.gpsimd.indirect_dma_start(
                            out=zg[:, gt, :],
                            out_offset=None,
                            in_=z_d[:, :],
                            in_offset=IndirectOffsetOnAxis(ap=izt[:, gt : gt + 1], axis=0),
                        )
                    # er expansion: per block gather the <=sw dst rows' er
                    # values, then scatter them onto edge slots with the
                    # transposed one-hot via TensorE
                    erg = epool.tile([P, gpt, kh], bf, name="erg", tag="erg")
                    for b in range(bg):
                        erb = epool.tile([sw, kh], bf, name="erb", tag="erb")
                        nc.gpsimd.indirect_dma_start(
                            out=erb[:],
                            out_offset=None,
                            in_=z_d[:, :],
                            in_offset=IndirectOffsetOnAxis(
                                ap=gdt[:sw, b : b + 1], axis=0
                            ),
                            element_offset=zgc,
                        )
                        erps = ppool.tile(
                            [P, tpb * kh], f32, name="erps", tag="erps", bufs=2
                        )
                        for t in range(tpb):
                            nc.tensor.matmul(
                                erps[:, t * kh : (t + 1) * kh],
                                lhsT=stsb[:, (b * tpb + t) * P : (b * tpb + t + 1) * P],
                                rhs=erb[:],
                                start=True,
                                stop=True,
                            )
                        nc.scalar.copy(
                            bass.AP(
                                erg.tensor,
                                erg.offset + b * tpb * kh,
                                [[gpt * kh, P], [1, tpb * kh]],
                            ),
                            erps[:],
                        )

                    zg_t = zg.tensor
                    zg_off = zg.offset

                    # logits x = el + er + c
                    x = epool.tile([P, gpt * kh], bf, name="x", tag="x")
                    el_ap = bass.AP(zg_t, zg_off + c, [[gpt * zgc, P], [zgc, gpt], [1, kh]])
                    nc.vector.tensor_tensor(
                        out=x[:], in0=el_ap, in1=erg[:], op=mybir.AluOpType.add
                    )
                    c_ap = bass.AP(
                        crep_sb.tensor, crep_sb.offset, [[kh, P], [0, gpt], [1, kh]]
                    )
                    nc.vector.tensor_tensor(
                        out=x[:], in0=x[:], in1=c_ap, op=mybir.AluOpType.add
                    )
                    # w = exp(leakyrelu(x)) = max(exp(x), exp(neg*x))
                    e1 = epool.tile([P, gpt * kh], bf, name="e1", tag="e1")
                    e2 = epool.tile([P, gpt * kh], bf, name="e2", tag="e2")
                    nc.scalar.activation(e1[:], x[:], mybir.ActivationFunctionType.Exp)
                    nc.scalar.activation(
                        e2[:], x[:], mybir.ActivationFunctionType.Exp,
                        scale=float(cfg.neg_slope),
                    )
                    msb = epool.tile([P, gpt, zgc], bf, name="msb", tag="msb")
                    m_t = msb.tensor
                    m_off = msb.offset
                    w_ap = bass.AP(m_t, m_off + c, [[gpt * zgc, P], [zgc, gpt], [1, kh]])
                    nc.vector.tensor_tensor(
                        out=w_ap, in0=e1[:], in1=e2[:], op=mybir.AluOpType.max
                    )
                    # weighted messages M = hs * w (head-minor broadcast)
                    m_out = bass.AP(m_t, m_off, [[gpt * zgc, P], [zgc, gpt], [1, c]])
                    hs_ap = bass.AP(zg_t, zg_off, [[gpt * zgc, P], [zgc, gpt], [1, c]])
                    wb_ap = bass.AP(
                        m_t, m_off + c, [[gpt * zgc, P], [zgc, gpt], [0, cfg.dh], [1, kh]]
                    )
                    nc.vector.tensor_tensor(
                        out=m_out, in0=hs_ap, in1=wb_ap, op=mybir.AluOpType.mult
                    )

                    # segment-sum matmuls + per-block flush
                    stg = epool.tile([sw, bg, zgc], f32, name="stg", tag="stg")
                    for b in range(bg):
                        pb = ppool.tile([sw, zgc], f32, name="pb", tag="pb", bufs=4)
                        for t in range(tpb):
                            nc.tensor.matmul(
                                pb[:, :],
                                lhsT=ssb[:, (b * tpb + t) * sw : (b * tpb + t + 1) * sw],
                                rhs=msb[:, b * tpb + t, :],
                                start=(t == 0),
                                stop=(t == tpb - 1),
                            )
                        nc.scalar.copy(stg[:, b, :], pb[:, :])

                    st_t = stg.tensor
                    st_off = stg.offset
                    den = bass.AP(st_t, st_off + c, [[bg * zgc, sw], [zgc, bg], [1, kh]])
                    nc.vector.tensor_scalar_max(den, den, 1e-30)
                    rcp = epool.tile([sw, bg * kh], f32, name="rcp", tag="rcp")
                    nc.vector.reciprocal(rcp[:], den)

                    outp = epool.tile([sw, bg, c], f32, name="outp", tag="outp")
                    o_t = outp.tensor
                    o_off = outp.offset
                    # numerator (head-minor d*kh+h) -> out col h*dh+d (head-major)
                    num_ap = bass.AP(
                        st_t, st_off, [[bg * zgc, sw], [zgc, bg], [kh, cfg.dh], [1, kh]]
                    )
                    out_ap = bass.AP(
                        o_t, o_off, [[bg * c, sw], [c, bg], [1, cfg.dh], [cfg.dh, kh]]
                    )
                    rcp_ap = bass.AP(
                        rcp.tensor, rcp.offset, [[bg * kh, sw], [kh, bg], [0, cfg.dh], [1, kh]]
                    )
                    nc.vector.tensor_tensor(
                        out=out_ap, in0=num_ap, in1=rcp_ap, op=mybir.AluOpType.mult
                    )
                    b_ap = bass.AP(
                        brep_sb.tensor, brep_sb.offset, [[c, sw], [0, bg], [1, c]]
                    )
                    ofl = bass.AP(o_t, o_off, [[bg * c, sw], [1, bg * c]])
                    nc.vector.tensor_tensor(
                        out=ofl, in0=ofl, in1=b_ap, op=mybir.AluOpType.add
                    )
                    for b in range(bg):
                        nc.gpsimd.indirect_dma_start(
                            out=out_d[:, :],
                            out_offset=IndirectOffsetOnAxis(
                                ap=oit[:sw, b : b + 1], axis=0
                            ),
                            in_=outp[:, b, :],
                            in_offset=None,
                        )

    nc.compile()
    return nc


# ----------------------------------------------------------------------------
# Entry point
# ----------------------------------------------------------------------------

def _run(cfg, inputs, trace=False):
    feat_src = np.asarray(inputs["feat_src"], dtype=np.float32)
    feat_dst = np.asarray(inputs["feat_dst"], dtype=np.float32)
    W_src = np.asarray(inputs["W_src"], dtype=np.float32)
    b_src = np.asarray(inputs["b_src"], dtype=np.float32)
    W_dst = np.asarray(inputs["W_dst"], dtype=np.float32)
    b_dst = np.asarray(inputs["b_dst"], dtype=np.float32)
    attn = np.asarray(inputs["attn"], dtype=np.float32)
    src_idx = np.asarray(inputs["src_idx"]).astype(np.int64)
    dst_idx = np.asarray(inputs["dst_idx"]).astype(np.int64)

    Wz, Wer, c_rep, b_rep = build_weights(cfg, W_src, b_src, W_dst, b_dst, attn)
    idx_z, idx_er, s_arr, st_arr, gdst, out_idx, meta = build_schedule(cfg, src_idx, dst_idx)
    nb, nbg, rng = meta["nb"], meta["nbg"], meta["rng"]

    fsrcT = np.ascontiguousarray(
        feat_src.T.reshape(cfg.kc, P, cfg.n_nodes)
    ).astype(BF16)
    fdstT = np.ascontiguousarray(
        feat_dst.T.reshape(cfg.kc, P, cfg.n_nodes)
    ).astype(BF16)

    nc = build_program(cfg, nb, nbg, rng)

    in_maps = []
    for ci in range(cfg.n_cores):
        in_maps.append(
            {
                "fsrcT": fsrcT,
                "fdstT": fdstT,
                "wz": Wz,
                "wer": Wer,
                "crep": c_rep,
                "brep": b_rep,
                "idxz": idx_z[ci],
                "gdst": gdst[ci],
                "s": s_arr[ci],
                "st": st_arr[ci],
                "oidx": out_idx[ci],
            }
        )

    res = run_bass_kernel_spmd(
        nc, in_maps, core_ids=list(range(cfg.n_cores)), trace=trace
    )

    bounds = meta["bounds"]
    out = np.empty((cfg.n_nodes, cfg.c), dtype=np.float32)
    for ci in range(cfg.n_cores):
        d0, d1 = bounds[ci], bounds[ci + 1]
        out[d0:d1] = res.results[ci]["out"][: d1 - d0]
    # zero-in-degree nodes aggregate nothing (the softmax-weights-sum-to-1
    # bias fold only holds when there is at least one incoming edge)
    deg = np.bincount(dst_idx, minlength=cfg.n_nodes)
    out[deg == 0] = 0.0
    return out, res


def kernel(**inputs) -> np.ndarray:
    out, _ = _run(FULL_CFG, inputs, trace=False)
    return out


# revision 21
# speedup vs baseline: 1.4741x; 1.1590x over previous
"""GAT-style message passing (edge softmax + weighted aggregation) on 8 NeuronCores.

Algorithm (mathematically identical to the reference, up to fp reassociation):
  hs = feat_src @ W_src.T + b_src            (per node, 8 heads x 16 dims)
  el = <hs, a_l>, er = <hd, a_r>             (per node, per head)
  w_e = exp(leakyrelu(el[src]+er[dst]))      (per edge; softmax max-subtraction
                                              dropped -- logits are O(1), exp is
                                              safe, and the max cancels exactly
                                              in the normalization)
  out[v] = (sum_e w_e * hs_nb[src]) / (sum_e w_e) + b_src
           (b_src factors out because softmax weights sum to 1 per (v, head))

Device mapping:
  - "Z table" in HBM per core: row n = [hs_nb (128 cols, head-minor) | el (8) | er (8)]
    built by a replicated bf16 GEMM over all nodes (TensorE), bias-free.
  - Edges sorted by dst; dst nodes split into 8 contiguous per-core ranges with
    ~equal edge counts.  Per core, consecutive dsts are greedily packed into
    blocks of <= SW dsts and <= TPB*128 edge slots.
  - Per 128-edge tile: rows of the Z table are fetched with indirect DMA
    (src-indexed 136-col reads, one row per partition per call -- the HW DGE
    contract); per-block er values are fetched once per block (<=64 rows) and
    expanded to edge slots via a transposed one-hot matmul on TensorE.
    Attention weights are computed on VectorE/ScalarE, and a host-built
    one-hot matrix S maps the weighted messages onto the block's dst slots
    via a TensorE matmul that accumulates in PSUM (segment-sum as matmul).
    The un-normalized softmax denominator rides along as 8 extra columns.
  - Per block: normalize (reciprocal + multiply, converting the head-minor
    layout back to head-major), add b_src, and scatter rows to the core's
    output range with indirect DMA.
"""

import sys

for _p in ("/opt/trn_rl_repo",):
    if _p not in sys.path:
        sys.path.insert(0, _p)

import numpy as np
import ml_dtypes

import concourse.bass as bass
import concourse.bacc as bacc
import concourse.mybir as mybir
import concourse.tile as tile
from concourse.bass import IndirectOffsetOnAxis
from concourse.bass_utils import run_bass_kernel_spmd

BF16 = ml_dtypes.bfloat16
P = 128  # SBUF partitions / edge slots per tile


class Cfg:
    def __init__(self, n_nodes, d_in, kh, dh, n_cores, sw, tpb, bg, neg_slope=0.2):
        assert d_in % P == 0
        self.n_nodes = n_nodes
        self.d_in = d_in
        self.kh = kh  # heads
        self.dh = dh  # dims per head
        self.c = kh * dh  # feature cols (128)
        self.n_cores = n_cores
        self.sw = sw  # max dsts per block (S width)
        self.tpb = tpb  # tiles (of 128 edge slots) per block
        self.slots = tpb * P
        self.bg = bg  # blocks per processing batch
        self.neg_slope = neg_slope
        self.kc = d_in // P  # K chunks for GEMM
        self.zgc = self.c + kh  # gathered cols per src row (hs + el)
        self.zcols = self.zgc + kh  # stored row cols (hs + el + er)
        self.zrow = 2 * self.c  # stored row stride (512B bf16, dma_gather granule)
        self.split = n_nodes // 2  # A half: nodes < split (int16 idx reach)
        self.tpa = tpb // 2  # A tiles per block (B gets the rest)
        assert tpb % 2 == 0 and self.split + 1 <= 32768 and (n_nodes + 2 - self.split - 1) <= 32768
        assert self.c == P, "flush/permute APs assume C == 128"


FULL_CFG = Cfg(n_nodes=50000, d_in=256, kh=8, dh=16, n_cores=8, sw=64, tpb=8, bg=4)


# ----------------------------------------------------------------------------
# Host-side preprocessing
# ----------------------------------------------------------------------------

def _head_minor_perm(cfg):
    # z col c (c < C) holds original channel (c % kh) * dh + (c // kh)
    c = np.arange(cfg.c)
    return (c % cfg.kh) * cfg.dh + (c // cfg.kh)


def build_weights(cfg, W_src, b_src, W_dst, b_dst, attn):
    kh, dh, din = cfg.kh, cfg.dh, cfg.d_in
    a_l = attn[:, :dh]  # (kh, dh)
    a_r = attn[:, dh:]
    perm = _head_minor_perm(cfg)

    W_el = np.einsum("kd,kdi->ki", a_l, W_src.reshape(kh, dh, din))  # (kh, din)
    W_er = np.einsum("kd,kdi->ki", a_r, W_dst.reshape(kh, dh, din))
    c_el = np.einsum("kd,kd->k", a_l, b_src.reshape(kh, dh))
    c_er = np.einsum("kd,kd->k", a_r, b_dst.reshape(kh, dh))

    # src-side rhs: cols [hs head-minor (C) | el (kh)]
    Wz = np.concatenate([W_src[perm].T, W_el.T], axis=1)  # (din, C+kh)
    Wz = Wz.reshape(cfg.kc, P, cfg.zgc).astype(BF16)
    Wer = W_er.T.reshape(cfg.kc, P, kh).astype(BF16)  # (kc, P, kh)

    c_vec = (c_el + c_er).astype(np.float32)
    c_rep = np.tile(c_vec[None, :], (P, 1)).astype(BF16)  # (P, kh)
    b_rep = np.tile(b_src[None, :], (P, 1)).astype(np.float32)  # (P, C) head-major
    return Wz, Wer, c_rep, b_rep


def build_schedule(cfg, src_idx, dst_idx):
    """Sort edges by dst, split dsts across cores, pack blocks, build per-core
    index/one-hot arrays (uniform shapes across cores)."""
    E = src_idx.shape[0]
    n = cfg.n_nodes
    order = np.argsort(dst_idx, kind="stable")
    ssrc = src_idx[order].astype(np.int64)
    counts = np.bincount(dst_idx, minlength=n).astype(np.int64)
    starts = np.zeros(n + 1, dtype=np.int64)
    np.cumsum(counts, out=starts[1:])

    # split dst range into n_cores contiguous chunks with ~equal edges
    bounds = [0]
    for ci in range(1, cfg.n_cores):
        target = E * ci // cfg.n_cores
        d = int(np.searchsorted(starts, target))
        d = max(bounds[-1], min(d, n))
        bounds.append(d)
    bounds.append(n)

    per_core_blocks = []  # list of list of (d0, ndst, edge_start, nedges)
    for ci in range(cfg.n_cores):
        d0, d1 = bounds[ci], bounds[ci + 1]
        blocks = []
        d = d0
        acnt = np.bincount(dst_idx[src_idx < cfg.split], minlength=n).astype(np.int64)
        cap = cfg.tpa * P
        while d < d1:
            bstart = d
            ua = ub = 0
            while (
                d < d1
                and (d - bstart) < cfg.sw
                and ua + acnt[d] <= cap
                and ub + (counts[d] - acnt[d]) <= cap
            ):
                ua += int(acnt[d])
                ub += int(counts[d] - acnt[d])
                d += 1
            assert d > bstart, f"dst {d} degree {counts[d]} exceeds block caps"
            blocks.append((bstart, d - bstart, int(starts[bstart]), ua + ub))
        per_core_blocks.append(blocks)

    nb = max(len(b) for b in per_core_blocks)
    nbg = -(-nb // cfg.bg)
    nb = nbg * cfg.bg
    nt = nb * cfg.tpb  # tiles per core

    rng = max(bounds[ci + 1] - bounds[ci] for ci in range(cfg.n_cores))
    trash = rng  # local trash row index in the output table

    # int16 gather indices, one flat list per (half, batch):
    # flat position i -> out slot (i % 128, i // 128); idx tile layout is
    # (128, ncols) with flat i at [i % 16, i // 16], replicated 8x across
    # the partition dim (one copy per Q7 core's 16-partition slice)
    nia = cfg.bg * cfg.tpa * P  # idxs per half-batch gather
    idxa = np.zeros((cfg.n_cores, nbg, nia), dtype=np.int16)
    idxb = np.zeros((cfg.n_cores, nbg, nia), dtype=np.int16)
    s_arr = np.zeros((cfg.n_cores, nb, cfg.tpb, P, cfg.sw), dtype=BF16)
    out_idx = np.full((cfg.n_cores, P, nb), trash, dtype=np.int32)
    gdst = np.zeros((cfg.n_cores, P, nb), dtype=np.int32)
    zrow_of = np.where(np.arange(n) < cfg.split, np.arange(n) + 1, np.arange(n) + 2)

    for ci in range(cfg.n_cores):
        d0c = bounds[ci]
        for bi, (bstart, ndst, estart, nedges) in enumerate(per_core_blocks[ci]):
            gi, bj = bi // cfg.bg, bi % cfg.bg
            if nedges > 0:
                esrc = ssrc[estart : estart + nedges]
                dloc = np.repeat(
                    np.arange(ndst), counts[bstart : bstart + ndst]
                ).astype(np.int64)
                isa = esrc < cfg.split
                for half, mask in ((0, isa), (1, ~isa)):
                    hsrc = esrc[mask]
                    hloc = dloc[mask]
                    sl = np.arange(len(hsrc))
                    t = half * cfg.tpa + sl // P
                    p = sl % P
                    s_arr[ci, bi, t, p, hloc] = 1.0
                    # flat gather position within the half-batch list
                    fp = (bj * cfg.tpa + sl // P) * P + p
                    loc = zrow_of[hsrc] - (0 if half == 0 else cfg.split + 1)
                    (idxa if half == 0 else idxb)[ci, gi, fp] = loc
            out_idx[ci, :ndst, bi] = np.arange(bstart - d0c, bstart - d0c + ndst)
            gdst[ci, :ndst, bi] = zrow_of[np.arange(bstart, bstart + ndst)]

    # wrap idx lists into the replicated (128, ncols) int16 tile layout
    def wrap16(a):
        # a: (n_cores, nbg, nia) -> (n_cores, nbg, 128, nia // 16)
        w = a.reshape(cfg.n_cores, nbg, nia // 16, 16).transpose(0, 1, 3, 2)
        return np.ascontiguousarray(np.tile(w, (1, 1, 8, 1)))

    idxa = wrap16(idxa)
    idxb = wrap16(idxb)

    # transposed one-hot for the er-expansion matmul: (n_cores, nbg, sw, bg*tpb*P)
    st_arr = np.ascontiguousarray(
        s_arr.reshape(cfg.n_cores, nbg, cfg.bg, cfg.tpb, P, cfg.sw)
        .transpose(0, 1, 5, 2, 3, 4)
        .reshape(cfg.n_cores, nbg, cfg.sw, cfg.bg * cfg.tpb * P)
    )
    # rearrange S to SBUF-friendly layout: (n_cores, nbg, P, bg*tpb*sw)
    s_arr = (
        s_arr.reshape(cfg.n_cores, nbg, cfg.bg, cfg.tpb, P, cfg.sw)
        .transpose(0, 1, 4, 2, 3, 5)
        .reshape(cfg.n_cores, nbg, P, cfg.bg * cfg.tpb * cfg.sw)
    )
    s_arr = np.ascontiguousarray(s_arr)
    meta = dict(bounds=bounds, nb=nb, nbg=nbg, nt=nt, rng=rng)
    return idxa, idxb, s_arr, st_arr, gdst, out_idx, meta


# ----------------------------------------------------------------------------
# Device program
# ----------------------------------------------------------------------------

def build_program(cfg, nb, nbg, rng_sz):
    n, kh, c, kc = cfg.n_nodes, cfg.kh, cfg.c, cfg.kc
    zgc, zcols, sw, tpb, bg = cfg.zgc, cfg.zcols, cfg.sw, cfg.tpb, cfg.bg
    zrow, split, tpa = cfg.zrow, cfg.split, cfg.tpa
    nt = nb * tpb
    gpt = bg * tpb  # gather tiles per batch
    hgt = bg * tpa  # gather tiles per half-batch
    nia = hgt * P  # idxs per half-batch dma_gather
    bf = mybir.dt.bfloat16
    f32 = mybir.dt.float32
    i32 = mybir.dt.int32

    nc = bacc.Bacc(
        "TRN2", target_bir_lowering=False, debug=False, num_devices=cfg.n_cores
    )

    fsrcT = nc.dram_tensor("fsrcT", [kc, P, n], bf, kind="ExternalInput")
    fdstT = nc.dram_tensor("fdstT", [kc, P, n], bf, kind="ExternalInput")
    wz_d = nc.dram_tensor("wz", [kc, P, zgc], bf, kind="ExternalInput")
    wer_d = nc.dram_tensor("wer", [kc, P, kh], bf, kind="ExternalInput")
    crep_d = nc.dram_tensor("crep", [P, kh], bf, kind="ExternalInput")
    brep_d = nc.dram_tensor("brep", [P, c], f32, kind="ExternalInput")
    i16 = mybir.dt.int16
    idxa_d = nc.dram_tensor("idxa", [nbg, P, nia // 16], i16, kind="ExternalInput")
    idxb_d = nc.dram_tensor("idxb", [nbg, P, nia // 16], i16, kind="ExternalInput")
    gdst_d = nc.dram_tensor("gdst", [P, nb], i32, kind="ExternalInput")
    s_d = nc.dram_tensor("s", [nbg, P, bg * tpb * sw], bf, kind="ExternalInput")
    st_d = nc.dram_tensor("st", [nbg, sw, bg * tpb * P], bf, kind="ExternalInput")
    oidx_d = nc.dram_tensor("oidx", [P, nb], i32, kind="ExternalInput")

    z_d = nc.dram_tensor("ztab", [n + 2, zrow], bf, kind="Internal")
    out_d = nc.dram_tensor("out", [rng_sz + 1, c], f32, kind="ExternalOutput")

    ntile_n = -(-n // P)

    with tile.TileContext(nc) as tc:
        with (
            tc.tile_pool(name="consts", bufs=1) as cpool,
            tc.tile_pool(name="psum", bufs=4, space="PSUM") as ppool,
        ):
            # ---------------- Phase A: node GEMMs -> Z table ----------------
            wz_sb = [
                cpool.tile([P, zgc], bf, name=f"wz_sb{k}", tag=f"wz_sb{k}")
                for k in range(kc)
            ]
            wer_sb = [
                cpool.tile([P, kh], bf, name=f"wer_sb{k}", tag=f"wer_sb{k}")
                for k in range(kc)
            ]
            crep_sb = cpool.tile([P, kh], bf, name="crep_sb")
            brep_sb = cpool.tile([P, c], f32, name="brep_sb")
            for k in range(kc):
                nc.sync.dma_start(out=wz_sb[k][:], in_=wz_d[k])
                nc.sync.dma_start(out=wer_sb[k][:], in_=wer_d[k])
            nc.sync.dma_start(out=crep_sb[:], in_=crep_d[:])
            nc.sync.dma_start(out=brep_sb[:], in_=brep_d[:])
            zz = cpool.tile([2, zrow], bf, name="zz")
            nc.gpsimd.memset(zz[:], 0)
            nc.sync.dma_start(out=z_d[0:1, :], in_=zz[0:1, :])
            nc.sync.dma_start(out=z_d[split + 1 : split + 2, :], in_=zz[1:2, :])

            with tc.tile_pool(name="gemm", bufs=3) as gpool:
                for it in range(ntile_n):
                    n0 = it * P
                    nn = min(P, n - n0)
                    fs = gpool.tile([P, kc, P], bf, name="fs", tag="fs")
                    fd = gpool.tile([P, kc, P], bf, name="fd", tag="fd")
                    # (P, kc, nn): partition = feat-dim chunk rows
                    nc.sync.dma_start(
                        out=fs[:, :, :nn],
                        in_=bass.AP(fsrcT.ap().tensor, n0, [[n, P], [P * n, kc], [1, nn]]),
                    )
                    nc.sync.dma_start(
                        out=fd[:, :, :nn],
                        in_=bass.AP(fdstT.ap().tensor, n0, [[n, P], [P * n, kc], [1, nn]]),
                    )
                    ps = ppool.tile([P, zcols], f32, name="ps", tag="ps", bufs=2)
                    for k in range(kc):
                        nc.tensor.matmul(
                            ps[:nn, 0:zgc],
                            lhsT=fs[:, k, :nn],
                            rhs=wz_sb[k][:],
                            start=(k == 0),
                            stop=(k == kc - 1),
                        )
                    for k in range(kc):
                        nc.tensor.matmul(
                            ps[:nn, zgc:zcols],
                            lhsT=fd[:, k, :nn],
                            rhs=wer_sb[k][:],
                            start=(k == 0),
                            stop=(k == kc - 1),
                        )
                    zsb = gpool.tile([P, zcols], bf, name="zsb", tag="zsb")
                    nc.scalar.copy(zsb[:nn], ps[:nn])
                    if n0 + nn <= split:
                        nc.sync.dma_start(
                            out=z_d[n0 + 1 : n0 + 1 + nn, 0:zcols], in_=zsb[:nn]
                        )
                    elif n0 >= split:
                        nc.sync.dma_start(
                            out=z_d[n0 + 2 : n0 + 2 + nn, 0:zcols], in_=zsb[:nn]
                        )
                    else:
                        na = split - n0
                        nc.sync.dma_start(
                            out=z_d[n0 + 1 : split + 1, 0:zcols], in_=zsb[:na]
                        )
                        nc.sync.dma_start(
                            out=z_d[split + 2 : n0 + 2 + nn, 0:zcols], in_=zsb[na:nn]
                        )

            tc.strict_bb_all_engine_barrier()

            # ---------------- Phase B: edge processing ----------------------
            with tc.tile_pool(name="edge", bufs=2) as epool:
                for g in range(nbg):
                    g0 = g * gpt  # first gather tile of batch
                    ia = epool.tile([P, nia // 16], i16, name="ia", tag="ia")
                    nc.sync.dma_start(out=ia[:], in_=idxa_d[g])
                    ib = epool.tile([P, nia // 16], i16, name="ib", tag="ib")
                    nc.sync.dma_start(out=ib[:], in_=idxb_d[g])
                    ssb = epool.tile([P, bg * tpb * sw], bf, name="ssb", tag="ssb")
                    nc.sync.dma_start(out=ssb[:], in_=s_d[g])
                    stsb = epool.tile([sw, bg * tpb * P], bf, name="stsb", tag="stsb")
                    nc.sync.dma_start(out=stsb[:], in_=st_d[g])
                    oit = epool.tile([P, bg], i32, name="oit", tag="oit")
                    nc.sync.dma_start(out=oit[:], in_=oidx_d[:, g * bg : (g + 1) * bg])
                    gdt = epool.tile([P, bg], i32, name="gdt", tag="gdt")
                    nc.sync.dma_start(out=gdt[:], in_=gdst_d[:, g * bg : (g + 1) * bg])

                    # batched MoE-style row gather from each table half,
                    # in sub-calls of 512 idxs
                    sub = min(1024, nia)
                    nsub = nia // sub
                    zga = epool.tile([P, hgt, zrow], bf, name="zga", tag="zga")
                    zgb = epool.tile([P, hgt, zrow], bf, name="zgb", tag="zgb")
                    for half, zgh, ixt in ((0, zga, ia), (1, zgb, ib)):
                        src_ap = (
                            z_d[0 : split + 1, :] if half == 0 else z_d[split + 1 : n + 2, :]
                        )
                        for k in range(nsub):
                            nc.gpsimd.dma_gather(
                                out_ap=zgh[:, k * (sub // P) : (k + 1) * (sub // P), :],
                                in_ap=src_ap,
                                idxs_ap=ixt[:, k * (sub // 16) : (k + 1) * (sub // 16)],
                                num_idxs=sub,
                                num_idxs_reg=sub,
                                elem_size=zrow,
                            )
                    # er expansion: per block gather the <=sw dst rows' er
                    # values, then scatter them onto edge slots with the
                    # transposed one-hot via TensorE
                    erg = epool.tile([P, gpt, kh], bf, name="erg", tag="erg")
                    for b in range(bg):
                        erb = epool.tile([sw, kh], bf, name="erb", tag="erb")
                        nc.gpsimd.indirect_dma_start(
                            out=erb[:],
                            out_offset=None,
                            in_=z_d[:, :],
                            in_offset=IndirectOffsetOnAxis(
                                ap=gdt[:sw, b : b + 1], axis=0
                            ),
                            element_offset=zgc,
                        )
                        erps = ppool.tile(
                            [P, tpb * kh], f32, name="erps", tag="erps", bufs=2
                        )
                        for t in range(tpb):
                            nc.tensor.matmul(
                                erps[:, t * kh : (t + 1) * kh],
                                lhsT=stsb[:, (b * tpb + t) * P : (b * tpb + t + 1) * P],
                                rhs=erb[:],
                                start=True,
                                stop=True,
                            )
                        nc.scalar.copy(
                            bass.AP(
                                erg.tensor,
                                erg.offset + b * tpb * kh,
                                [[gpt * kh, P], [1, tpb * kh]],
                            ),
                            erps[:],
                        )

                    # logits x = el + er + c (A and B halves separately)
                    x = epool.tile([P, gpt * kh], bf, name="x", tag="x")
                    for half, zgh in ((0, zga), (1, zgb)):
                        ho = half * tpa
                        x_ap = bass.AP(
                            x.tensor, x.offset + ho * kh,
                            [[gpt * kh, P], [tpb * kh, bg], [kh, tpa], [1, kh]],
                        )
                        el_ap = bass.AP(
                            zgh.tensor, zgh.offset + c,
                            [[hgt * zrow, P], [tpa * zrow, bg], [zrow, tpa], [1, kh]],
                        )
                        er_ap = bass.AP(
                            erg.tensor, erg.offset + ho * kh,
                            [[gpt * kh, P], [tpb * kh, bg], [kh, tpa], [1, kh]],
                        )
                        nc.vector.tensor_tensor(
                            out=x_ap, in0=el_ap, in1=er_ap, op=mybir.AluOpType.add
                        )
                    c_ap = bass.AP(
                        crep_sb.tensor, crep_sb.offset, [[kh, P], [0, gpt], [1, kh]]
                    )
                    nc.vector.tensor_tensor(
                        out=x[:], in0=x[:], in1=c_ap, op=mybir.AluOpType.add
                    )
                    # w = exp(leakyrelu(x)) = max(exp(x), exp(neg*x))
                    e1 = epool.tile([P, gpt * kh], bf, name="e1", tag="e1")
                    e2 = epool.tile([P, gpt * kh], bf, name="e2", tag="e2")
                    nc.scalar.activation(e1[:], x[:], mybir.ActivationFunctionType.Exp)
                    nc.scalar.activation(
                        e2[:], x[:], mybir.ActivationFunctionType.Exp,
                        scale=float(cfg.neg_slope),
                    )
                    msb = epool.tile([P, gpt, zgc], bf, name="msb", tag="msb")
                    m_t = msb.tensor
                    m_off = msb.offset
                    w_ap = bass.AP(m_t, m_off + c, [[gpt * zgc, P], [zgc, gpt], [1, kh]])
                    nc.vector.tensor_tensor(
                        out=w_ap, in0=e1[:], in1=e2[:], op=mybir.AluOpType.max
                    )
                    # weighted messages M = hs * w (head-minor broadcast),
                    # per (block, half) to stay within 4 AP dims
                    for b in range(bg):
                        for half, zgh in ((0, zga), (1, zgb)):
                            mo = b * tpb * zgc + half * tpa * zgc
                            m_out = bass.AP(
                                m_t, m_off + mo, [[gpt * zgc, P], [zgc, tpa], [1, c]]
                            )
                            hs_ap = bass.AP(
                                zgh.tensor, zgh.offset + b * tpa * zrow,
                                [[hgt * zrow, P], [zrow, tpa], [1, c]],
                            )
                            wb_ap = bass.AP(
                                m_t, m_off + mo + c,
                                [[gpt * zgc, P], [zgc, tpa], [0, cfg.dh], [1, kh]],
                            )
                            nc.vector.tensor_tensor(
                                out=m_out, in0=hs_ap, in1=wb_ap, op=mybir.AluOpType.mult
                            )

                    # segment-sum matmuls + per-block flush
                    stg = epool.tile([sw, bg, zgc], f32, name="stg", tag="stg")
                    for b in range(bg):
                        pb = ppool.tile([sw, zgc], f32, name="pb", tag="pb", bufs=4)
                        for t in range(tpb):
                            nc.tensor.matmul(
                                pb[:, :],
                                lhsT=ssb[:, (b * tpb + t) * sw : (b * tpb + t + 1) * sw],
                                rhs=msb[:, b * tpb + t, :],
                                start=(t == 0),
                                stop=(t == tpb - 1),
                            )
                        nc.scalar.copy(stg[:, b, :], pb[:, :])

                    st_t = stg.tensor
                    st_off = stg.offset
                    den = bass.AP(st_t, st_off + c, [[bg * zgc, sw], [zgc, bg], [1, kh]])
                    nc.vector.tensor_scalar_max(den, den, 1e-30)
                    rcp = epool.tile([sw, bg * kh], f32, name="rcp", tag="rcp")
                    nc.vector.reciprocal(rcp[:], den)

                    outp = epool.tile([sw, bg, c], f32, name="outp", tag="outp")
                    o_t = outp.tensor
                    o_off = outp.offset
                    # numerator (head-minor d*kh+h) -> out col h*dh+d (head-major)
                    num_ap = bass.AP(
                        st_t, st_off, [[bg * zgc, sw], [zgc, bg], [kh, cfg.dh], [1, kh]]
                    )
                    out_ap = bass.AP(
                        o_t, o_off, [[bg * c, sw], [c, bg], [1, cfg.dh], [cfg.dh, kh]]
                    )
                    rcp_ap = bass.AP(
                        rcp.tensor, rcp.offset, [[bg * kh, sw], [kh, bg], [0, cfg.dh], [1, kh]]
                    )
                    nc.vector.tensor_tensor(
                        out=out_ap, in0=num_ap, in1=rcp_ap, op=mybir.AluOpType.mult
                    )
                    b_ap = bass.AP(
                        brep_sb.tensor, brep_sb.offset, [[c, sw], [0, bg], [1, c]]
                    )
                    ofl = bass.AP(o_t, o_off, [[bg * c, sw], [1, bg * c]])
                    nc.vector.tensor_tensor(
                        out=ofl, in0=ofl, in1=b_ap, op=mybir.AluOpType.add
                    )
                    for b in range(bg):
                        nc.gpsimd.indirect_dma_start(
                            out=out_d[:, :],
                            out_offset=IndirectOffsetOnAxis(
                                ap=oit[:sw, b : b + 1], axis=0
                            ),
                            in_=outp[:, b, :],
                            in_offset=None,
                        )

    nc.compile()
    return nc


# ----------------------------------------------------------------------------
# Entry point
# ----------------------------------------------------------------------------

def _run(cfg, inputs, trace=False):
    feat_src = np.asarray(inputs["feat_src"], dtype=np.float32)
    feat_dst = np.asarray(inputs["feat_dst"], dtype=np.float32)
    W_src = np.asarray(inputs["W_src"], dtype=np.float32)
    b_src = np.asarray(inputs["b_src"], dtype=np.float32)
    W_dst = np.asarray(inputs["W_dst"], dtype=np.float32)
    b_dst = np.asarray(inputs["b_dst"], dtype=np.float32)
    attn = np.asarray(inputs["attn"], dtype=np.float32)
    src_idx = np.asarray(inputs["src_idx"]).astype(np.int64)
    dst_idx = np.asarray(inputs["dst_idx"]).astype(np.int64)

    Wz, Wer, c_rep, b_rep = build_weights(cfg, W_src, b_src, W_dst, b_dst, attn)
    idxa, idxb, s_arr, st_arr, gdst, out_idx, meta = build_schedule(cfg, src_idx, dst_idx)
    nb, nbg, rng = meta["nb"], meta["nbg"], meta["rng"]

    fsrcT = np.ascontiguousarray(
        feat_src.T.reshape(cfg.kc, P, cfg.n_nodes)
    ).astype(BF16)
    fdstT = np.ascontiguousarray(
        feat_dst.T.reshape(cfg.kc, P, cfg.n_nodes)
    ).astype(BF16)

    nc = build_program(cfg, nb, nbg, rng)

    in_maps = []
    for ci in range(cfg.n_cores):
        in_maps.append(
            {
                "fsrcT": fsrcT,
                "fdstT": fdstT,
                "wz": Wz,
                "wer": Wer,
                "crep": c_rep,
                "brep": b_rep,
                "idxa": idxa[ci],
                "idxb": idxb[ci],
                "gdst": gdst[ci],
                "s": s_arr[ci],
                "st": st_arr[ci],
                "oidx": out_idx[ci],
            }
        )

    res = run_bass_kernel_spmd(
        nc, in_maps, core_ids=list(range(cfg.n_cores)), trace=trace
    )

    bounds = meta["bounds"]
    out = np.empty((cfg.n_nodes, cfg.c), dtype=np.float32)
    for ci in range(cfg.n_cores):
        d0, d1 = bounds[ci], bounds[ci + 1]
        out[d0:d1] = res.results[ci]["out"][: d1 - d0]
    # zero-in-degree nodes aggregate nothing (the softmax-weights-sum-to-1
    # bias fold only holds when there is at least one incoming edge)
    deg = np.bincount(dst_idx, minlength=cfg.n_nodes)
    out[deg == 0] = 0.0
    return out, res


def kernel(**inputs) -> np.ndarray:
    out, _ = _run(FULL_CFG, inputs, trace=False)
    return out
